# revision 25
# baseline (speedup 1.0000x reference)
"""GQA attention kernel for Trainium2, 8 NeuronCores (v2, fp16).

Problem: B=2, T=2048, D=1024, 16 Q heads / 4 KV heads, head_dim=64, RoPE,
causal softmax, out-projection.

Sharding: 8 cores = 2 (batch) x 4 (KV group). Core c handles batch c//4 and
KV group g=c%4 (query heads 4g..4g+3). wq/wk/wv column-sharded, wo
row-sharded; the 4 partial outputs per batch are summed on the host.

v2 changes vs the fp32r baseline:
  * All matmul operands are fp16 (PSUM accumulation stays fp32).  fp32r
    matmuls measured ~3 cycles/row on HW and block LDWEIGHTS overlap (no
    FWL for fp32); fp16 runs 1 row/cycle with fast weight load.
  * Causal windows are shifted per key-block: for key block tj the query
    range is [128*tj, T) instead of 512-aligned chunks, trimming ~12% of
    score/PV rows, and the mask becomes a single static j<=i pattern
    applied with affine_select on the (otherwise idle) Pool engine after
    exp (multiplicative 0-fill on the first 128 columns only).
  * Queries processed in two 1024-halves; per (head, half, tj) ONE wide
    exp instruction covers the whole window (up to 1024 cols spanning two
    PSUM banks), halving ACT's ~350-cycle/instruction overhead count.
  * Softmax denominator: ones-columns ride in the PV stationary operand
    (col 0 and col 65 of v), so even heads get L at partition 64 and odd
    heads at partition 63 with pv rows on their natural ao partitions --
    no partition-base-shifting DMAs for ao assembly.
  * 1/L: L-row is staged to SBUF (fp16), DMA-transposed to [128, 8],
    reciprocal'd with full lane parallelism (the baseline burned 52us
    doing [1,512] reciprocals on one DVE lane), DMA'd back to a row and
    broadcast to 64 partitions with log2 doubling DMAs.
  * Projections for the second query half and the first half of the out
    projection are interleaved into the ACT-bound attention phases.
"""

import numpy as np
import sys

sys.path.insert(0, "/opt/trn_rl_repo")

from concourse import bass, bacc, mybir, tile  # noqa: E402
from concourse.bass_utils import run_bass_kernel_spmd  # noqa: E402

F32 = mybir.dt.float32
F16 = mybir.dt.float16

B, T, D = 2, 2048, 1024
HD = 64                      # head dim
NQH = 4                      # query heads per core
QCOLS = NQH * HD             # 256
KC = D // 128                # 8 contraction chunks
N_CORES = 8

_cache = {}


def _chunks512(a, b):
    """Split [a, b) at multiples of 512."""
    out = []
    while a < b:
        nxt = min(b, (a // 512 + 1) * 512)
        out.append((a, nxt))
        a = nxt
    return out


def build_nc():
    """Build the (SPMD-identical) single-core bass program."""
    nc = bacc.Bacc("TRN2", target_bir_lowering=False, debug=False)

    xT_d = nc.declare_dram_parameter("xT", [D, T], F16, isOutput=False)
    wq_d = nc.declare_dram_parameter("wq", [D, QCOLS], F16, isOutput=False)
    wk_d = nc.declare_dram_parameter("wk", [D, HD], F16, isOutput=False)
    wv_d = nc.declare_dram_parameter("wv", [D, HD], F16, isOutput=False)
    wo_d = nc.declare_dram_parameter("wo", [QCOLS, D], F16, isOutput=False)
    cos_d = nc.declare_dram_parameter("cosf", [128, T], F16, isOutput=False)
    sin_d = nc.declare_dram_parameter("sinf", [128, T], F16, isOutput=False)
    out_d = nc.declare_dram_parameter("out", [T, D], F32, isOutput=True)

    EXP = mybir.ActivationFunctionType.Exp

    with tile.TileContext(nc) as tc:
        with (
            tc.tile_pool(name="sb", bufs=1) as sb,
            tc.tile_pool(name="rotp", bufs=2) as rotp,
            tc.tile_pool(name="atp", bufs=3) as atp,
            tc.tile_pool(name="stgp", bufs=2) as stgp,
            tc.tile_pool(name="lcp", bufs=2) as lcp,
            tc.tile_pool(name="lip", bufs=2) as lip,
            tc.tile_pool(name="bcp", bufs=2) as bcp,
            tc.tile_pool(name="otp", bufs=3) as otp,
            tc.tile_pool(name="pp", bufs=2, space="PSUM") as pp,
            tc.tile_pool(name="scp", bufs=2, space="PSUM") as scp,
            tc.tile_pool(name="pvp", bufs=1, space="PSUM") as pvp,
        ):
            wq_s = sb.tile([128, KC, QCOLS], F16, tag="wq")
            wk_s = sb.tile([128, KC, HD], F16, tag="wk")
            wv_s = sb.tile([128, KC, HD], F16, tag="wv")
            wo_s = sb.tile([128, 2, D], F16, tag="wo")
            cosf = sb.tile([128, T], F16, tag="cosf")
            sinf = sb.tile([128, T], F16, tag="sinf")
            xTs = sb.tile([128, KC, T], F16, tag="xT")
            qT = [sb.tile([128, T], F16, tag=f"qT{hp}", name=f"qT{hp}")
                  for hp in range(2)]
            # kT duplicated into both partition halves so scores matmuls can
            # read it at base partition 0 (even heads) or 64 (odd heads).
            kT = sb.tile([128, T], F16, tag="kT")
            # v columns: 0 = ones, 1..64 = v channels, 65 = ones.  Even heads
            # use cols 1:66 (L lands at out-partition 64); odd heads use cols
            # 0:65 at out base 63 (L at 63, pv at 64:128).
            v = sb.tile([128, 16, HD + 2], F16, tag="v")
            ao = [sb.tile([128, T], F16, tag=f"ao{hp}", name=f"ao{hp}")
                  for hp in range(2)]

            ones64 = sb.tile([1, 64], F16, tag="ones64")

            # DMA issue order tracks the projection loop's consumption order
            # (k-inner, ci-outer) so the PE starts ~immediately and the x
            # stream stays ahead of it.
            for k in range(KC):
                nc.sync.dma_start(wk_s[:, k, :], wk_d[k * 128:(k + 1) * 128, :])
            for ci in range(4):
                cs = slice(ci * 512, (ci + 1) * 512)
                for k in range(KC):
                    nc.sync.dma_start(xTs[:, k, cs],
                                      xT_d[k * 128:(k + 1) * 128, cs])
                if ci == 0:
                    for k in range(KC):
                        nc.sync.dma_start(wq_s[:, k, :],
                                          wq_d[k * 128:(k + 1) * 128, :])
                        nc.sync.dma_start(wv_s[:, k, :],
                                          wv_d[k * 128:(k + 1) * 128, :])
                if ci == 1:
                    nc.sync.dma_start(cosf[:], cos_d[:])
                    nc.sync.dma_start(sinf[:], sin_d[:])
                if ci == 2:
                    for c in range(2):
                        nc.sync.dma_start(wo_s[:, c, :],
                                          wo_d[c * 128:(c + 1) * 128, :])
            nc.gpsimd.memset(v[:, :, 0:1], 1.0)
            nc.gpsimd.memset(v[:, :, HD + 1:HD + 2], 1.0)
            nc.gpsimd.memset(ones64[:, :], 1.0)

            # Warm-up: the PE clock gate (HAM) defaults to 1.2 GHz and only
            # releases to 2.4 GHz after ~3.4us of sustained activity.  While
            # the input DMAs stream in, run throwaway rank-1 matmuls so the
            # projections start at full clock.
            warm_src = sb.tile([1, 512], F16, tag="warm")
            nc.gpsimd.memset(warm_src[:, :], 1.0)
            for i in range(20):
                wt = pp.tile([128, 512], F32, tag="proj", name=f"warm{i}")
                nc.tensor.matmul(wt[0:64, :], ones64[0:1, :], warm_src[0:1, :],
                                 start=True, stop=True)

            def proj_piece_k(ci, hook=None):
                cs = slice(ci * 512, (ci + 1) * 512)
                pk = pp.tile([128, 512], F32, tag="proj", name=f"pk{ci}")
                for k in range(KC):
                    nc.tensor.matmul(pk[0:64, :], wk_s[:, k, :], xTs[:, k, cs],
                                     start=(k == 0), stop=(k == KC - 1))
                nc.vector.tensor_copy(kT[0:64, cs], pk[0:64, :])
                if hook:
                    hook()

            def proj_piece_q(ci, hp, hook=None):
                cs = slice(ci * 512, (ci + 1) * 512)
                pq = pp.tile([128, 512], F32, tag="proj", name=f"pq{ci}{hp}")
                for k in range(KC):
                    nc.tensor.matmul(
                        pq[:, :], wq_s[:, k, hp * 128:(hp + 1) * 128],
                        xTs[:, k, cs], start=(k == 0), stop=(k == KC - 1))
                nc.vector.tensor_copy(qT[hp][:, cs], pq[:, :])
                if hook:
                    hook()
                for t in (4 * ci + 2 * hp, 4 * ci + 2 * hp + 1):
                    pvt = pp.tile([128, 512], F32, tag="proj", name=f"pvt{t}")
                    for k in range(KC):
                        nc.tensor.matmul(
                            pvt[:, 0:HD],
                            xTs[:, k, t * 128:(t + 1) * 128],
                            wv_s[:, k, :],
                            start=(k == 0), stop=(k == KC - 1))
                    # ACT is idle during the leading projections (keeps DVE
                    # free for qT/kT copies + RoPE); chunks 2/3 run inside
                    # the exp-heavy attention phase, so use DVE there
                    # instead of queueing behind exps.
                    if ci < 2:
                        nc.scalar.copy(v[:, t, 1:HD + 1], pvt[:, 0:HD])
                    else:
                        nc.vector.tensor_copy(v[:, t, 1:HD + 1], pvt[:, 0:HD])

            def proj_chunk(ci, hooks=None):
                hooks = hooks or {}
                proj_piece_k(ci, hooks.get("k"))
                proj_piece_q(ci, 0, hooks.get("q0"))
                proj_piece_q(ci, 1, hooks.get("q1"))

            def rope(dst, nrows, cs):
                """dst = dst*cos + rot_half(dst)*sin on columns cs.

                The rotate-half partition swap is 2 interleaved-partition DMAs
                issued from the (idle) scalar DGE queue to dodge the loaded
                sync queue.
                """
                w = cs.stop - cs.start
                rot = rotp.tile([128, 1024], F16, tag="rot", name="rot")
                for blk in range(nrows // 64):
                    r0 = blk * 64
                    nc.scalar.dma_start(rot[r0:r0 + 32, 0:w],
                                        dst[r0 + 32:r0 + 64, cs])
                    nc.scalar.dma_start(rot[r0 + 32:r0 + 64, 0:w],
                                        dst[r0:r0 + 32, cs])
                nc.vector.tensor_mul(dst[0:nrows, cs], dst[0:nrows, cs],
                                     cosf[0:nrows, cs])
                nc.vector.tensor_mul(rot[0:nrows, 0:w], rot[0:nrows, 0:w],
                                     sinf[0:nrows, cs])
                nc.vector.tensor_add(dst[0:nrows, cs], dst[0:nrows, cs],
                                     rot[0:nrows, 0:w])

            def rope_hooks(qh):
                cs = slice(qh * 1024, (qh + 1) * 1024)

                def hk():
                    rope(kT, 64, cs)
                    nc.scalar.dma_start(kT[64:128, cs], kT[0:64, cs])

                return {
                    "k": hk,
                    "q0": lambda: rope(qT[0], 128, cs),
                    "q1": lambda: rope(qT[1], 128, cs),
                }

            def attn_head(qh, h, fillers=()):
                hp, hr = divmod(h, 2)
                qrow = slice(64 * hr, 64 * hr + 64)
                prow = slice(0, 65)
                vcols = slice(1, HD + 2)
                fillers = list(fillers)
                pv = pvp.tile([128, 1024], F32, tag="pv", name=f"pv{qh}{h}")

                def emit_pv(tj, ws, W, at):
                    # matmul outputs may not cross a PSUM bank boundary, so
                    # split at 512-multiples of the pv free index.
                    lo = ws - 1024 * qh
                    for (a, b) in _chunks512(lo, lo + W):
                        bk = a // 512
                        nc.tensor.matmul(
                            pv[prow, a:b], v[:, tj, vcols],
                            at[:, a - lo:b - lo],
                            start=(tj == 0),
                            stop=(tj == 8 * qh + 4 * bk + 3))

                prev = None
                for tj in range(8 * (qh + 1)):
                    ws = max(1024 * qh, 128 * tj)
                    W = 1024 * (qh + 1) - ws
                    sc = scp.tile([128, 1024], F32, tag="sc",
                                  name=f"sc{qh}{h}{tj}")
                    for (a, b) in _chunks512(0, W):
                        nc.tensor.matmul(
                            sc[:, a:b], kT[qrow, tj * 128:(tj + 1) * 128],
                            qT[hp][qrow, ws + a:ws + b],
                            start=True, stop=True)
                    at = atp.tile([128, 1024], F16, tag="at",
                                  name=f"at{qh}{h}{tj}")
                    nc.scalar.activation(at[:, 0:W], sc[:, 0:W], EXP,
                                         scale=0.125)
                    if ws == 128 * tj:  # window starts at the diagonal
                        nc.gpsimd.affine_select(
                            at[:, 0:128], at[:, 0:128],
                            pattern=[[1, 128]],
                            compare_op=mybir.AluOpType.is_ge,
                            fill=0.0, base=0, channel_multiplier=-1)
                    if prev is not None:
                        emit_pv(*prev)
                    # Sprinkle deferred filler work (proj pieces, out-proj
                    # tiles, normalize finishes) into the exp-bound loop so
                    # the PE never idles long enough for the HAM clock gate
                    # to re-throttle it.
                    if fillers and tj % 3 == 2:
                        fillers.pop(0)()
                    prev = (tj, ws, W, at)
                emit_pv(*prev)
                for fl in fillers:
                    fl()

                # normalize: stage pv (+L row) to SBUF fp16 (frees the pv psum
                # slot), transpose the L row via DMA, reciprocal on 128
                # lanes, DMA back to a row and broadcast to 64 partitions
                # with a rank-1 PE matmul into spare proj-psum slots.  The
                # broadcast + multiply are returned as a deferred closure so
                # the PE never stalls on the L DMA chain.  PE outputs must
                # start at partition 0/32/64, so all heads compute at base 0;
                # odd heads DMA the normalized result to ao partitions
                # 64:128 afterwards.
                stg = stgp.tile([128, 1024], F16, tag="stg", name=f"st{qh}{h}")
                nc.vector.tensor_copy(stg[prow, :], pv[prow, :])
                lc = lcp.tile([128, 8], F16, tag="lc", name=f"lc{qh}{h}")
                nc.scalar.dma_start(lc[:, :], stg[64:65, :])
                li = lip.tile([128, 8], F16, tag="li", name=f"li{qh}{h}")
                with nc.allow_low_precision(reason="fp16 1/L"):
                    nc.vector.reciprocal(li[:, :], lc[:, :])
                lr = bcp.tile([1, 1024], F16, tag="lr", name=f"lr{qh}{h}")
                nc.scalar.dma_start(lr[0:1, :], li[:, :])

                def finish():
                    half = slice(1024 * qh, 1024 * (qh + 1))
                    aot = None
                    if hr == 1:
                        aot = stgp.tile([128, 1024], F16, tag="aot",
                                        name=f"aot{qh}{h}")
                    for cc in range(2):
                        bc = pp.tile([128, 512], F32, tag="proj",
                                     name=f"bc{qh}{h}{cc}")
                        nc.tensor.matmul(bc[0:64, :], ones64[0:1, :],
                                         lr[0:1, cc * 512:(cc + 1) * 512],
                                         start=True, stop=True)
                        ccs = slice(cc * 512, (cc + 1) * 512)
                        if hr == 0:
                            dst = ao[hp][0:64, 1024 * qh + cc * 512:
                                         1024 * qh + (cc + 1) * 512]
                            nc.vector.tensor_mul(dst, stg[0:64, ccs],
                                                 bc[0:64, :])
                        else:
                            nc.vector.tensor_mul(aot[0:64, ccs],
                                                 stg[0:64, ccs], bc[0:64, :])
                    if hr == 1:
                        nc.scalar.dma_start(ao[hp][64:128, half], aot[0:64, :])

                return finish

            def outproj_tile(t, eng):
                ot = otp.tile([128, 1024], F32, tag="ot", name=f"ot{t}")
                for nh in range(2):
                    po = pp.tile([128, 512], F32, tag="proj",
                                 name=f"po{t}{nh}")
                    for cc in range(2):
                        nc.tensor.matmul(
                            po[:, :], ao[cc][:, t * 128:(t + 1) * 128],
                            wo_s[:, cc, nh * 512:(nh + 1) * 512],
                            start=(cc == 0), stop=(cc == 1))
                    dst = ot[:, nh * 512:(nh + 1) * 512]
                    if eng == "v":
                        nc.vector.tensor_copy(dst, po[:, :])
                    else:
                        nc.scalar.copy(dst, po[:, :])
                nc.sync.dma_start(out_d[t * 128:(t + 1) * 128, :], ot[:, :])

            # ---- schedule ----
            # Each attn_head returns a deferred normalize closure which is
            # passed as filler into a later head so the L-row DMA chain
            # never stalls the tensor engine.  The remaining projections and
            # the first half of the out-projection ride along as fillers
            # too, keeping the PE saturated through the exp-bound phases.
            # Odd heads (whose ao write needs an extra DMA hop) run first
            # within each half.
            hk1 = rope_hooks(1)
            proj_chunk(0)
            proj_chunk(1, rope_hooks(0))
            f01 = attn_head(0, 1, [
                lambda: proj_piece_k(2),
                lambda: proj_piece_q(2, 0),
                lambda: proj_piece_q(2, 1),
            ])
            f03 = attn_head(0, 3, [
                f01,
                lambda: proj_piece_k(3, hk1["k"]),
                lambda: proj_piece_q(3, 0, hk1["q0"]),
            ])
            f00 = attn_head(0, 0, [
                f03,
                lambda: proj_piece_q(3, 1, hk1["q1"]),
            ])
            f02 = attn_head(0, 2, [f00])
            f11 = attn_head(1, 1, [
                f02,
                lambda: outproj_tile(0, "v"),
                lambda: outproj_tile(1, "v"),
            ])
            f13 = attn_head(1, 3, [
                f11,
                lambda: outproj_tile(2, "v"),
                lambda: outproj_tile(3, "v"),
            ])
            f10 = attn_head(1, 0, [
                f13,
                lambda: outproj_tile(4, "v"),
                lambda: outproj_tile(5, "v"),
            ])
            f12 = attn_head(1, 2, [
                f10,
                lambda: outproj_tile(6, "s"),
                lambda: outproj_tile(7, "s"),
            ])
            f12()
            for t in range(8, 16):
                outproj_tile(t, "v" if t % 2 == 0 else "s")

    nc.compile()
    return nc


def make_in_maps(x, freqs_cos, freqs_sin, wq, wk, wv, wo):
    """Host-side sharding + layout prep. Returns per-core input dicts."""
    x = np.asarray(x, np.float32)
    fc = np.asarray(freqs_cos, np.float32)
    fs = np.asarray(freqs_sin, np.float32)
    wq = np.asarray(wq, np.float32)
    wk = np.asarray(wk, np.float32)
    wv = np.asarray(wv, np.float32)
    wo = np.asarray(wo, np.float32)

    perm = np.concatenate([np.arange(0, HD, 2), np.arange(1, HD, 2)])
    cosT = np.ascontiguousarray(fc.T)            # (32, T)
    sinT = np.ascontiguousarray(fs.T)
    cosf = np.concatenate([cosT] * 4, axis=0).astype(np.float16)
    sinf = np.concatenate([-sinT, sinT, -sinT, sinT], axis=0).astype(np.float16)

    in_maps = []
    for c in range(N_CORES):
        b, g = divmod(c, 4)
        wq_c = wq[:, g * QCOLS:(g + 1) * QCOLS]
        wq_c = np.ascontiguousarray(
            wq_c.reshape(D, NQH, HD)[:, :, perm].reshape(D, QCOLS))
        wk_c = np.ascontiguousarray(wk[:, g * HD:(g + 1) * HD][:, perm])
        wv_c = np.ascontiguousarray(wv[:, g * HD:(g + 1) * HD])
        wo_c = np.ascontiguousarray(wo[g * QCOLS:(g + 1) * QCOLS, :])
        xT_c = np.ascontiguousarray(x[b].T)
        in_maps.append({
            "xT": xT_c.astype(np.float16), "wq": wq_c.astype(np.float16),
            "wk": wk_c.astype(np.float16), "wv": wv_c.astype(np.float16),
            "wo": wo_c.astype(np.float16),
            "cosf": cosf, "sinf": sinf,
        })
    return in_maps


def run_on_cores(in_maps, trace=False, **kwargs):
    if "nc" not in _cache:
        _cache["nc"] = build_nc()
    return run_bass_kernel_spmd(
        _cache["nc"], in_maps, core_ids=list(range(N_CORES)), trace=trace,
        **kwargs)


def kernel(x, freqs_cos, freqs_sin, wq, wk, wv, wo):
    in_maps = make_in_maps(x, freqs_cos, freqs_sin, wq, wk, wv, wo)
    res = run_on_cores(in_maps)
    outs = [res.results[c]["out"] for c in range(N_CORES)]
    full = np.empty((B, T, D), np.float32)
    for b in range(B):
        full[b] = outs[4 * b] + outs[4 * b + 1] + outs[4 * b + 2] + outs[4 * b + 3]
    return full
